# revision 1
# baseline (speedup 1.0000x reference)
"""Trainium2 Bass kernel for nn_LoopVisibleLSTM (T=2048, B=32, D=256, H=256, L=2).

Architecture: the time recurrence is inherently sequential, so one core runs
the whole recurrence with the full batch (B=32).  Per 32-step half-block the
input projection x = input @ W_init.T + b_init is bulk-computed (PE
transposes + matmuls); per 4-step group the input-side gate projection
G0in = x @ Wih0.T + bias0 is bulk-matmul'ed directly into PSUM, packing the
4 steps x 32 batch rows into the 128 PSUM partitions.  Each sequential step
then issues one fp32r matmul (h.T stationary [128,32], Whh.T moving, N=512
chunks) that accumulates onto the bulk PSUM, activations on ACT, elementwise
on DVE, and PE transposes to produce the next step's h.T.
The "backward" half of the module's output is (faithfully to the reference)
just the final forward hidden state broadcast over time, assembled on host.
"""

import sys
import os

for _p in ("/opt/pypackages", "/opt/trn_rl_repo"):
    if _p not in sys.path:
        sys.path.insert(0, _p)

import numpy as np

T_FULL, B, D, H = 2048, 32, 256, 256
G4 = 4            # steps packed per PSUM group
HALF = 32         # steps per half-block (bulk x granularity)
BODY = 64         # steps per For_i body (2 half-blocks)
FP32 = None       # filled after imports
F32R = None


def build(T):
    """Build the Bass program for a T-step run. Returns nc."""
    import concourse.bass as bass
    import concourse.mybir as mybir
    import concourse.tile as tile
    from concourse import bacc
    from concourse.bass import AP  # noqa: F401

    global FP32, F32R
    FP32 = mybir.dt.float32
    F32R = mybir.dt.float32r
    AF = mybir.ActivationFunctionType

    assert T % BODY == 0
    n_body = T // BODY

    nc = bacc.Bacc("TRN2", target_bir_lowering=False, debug=False)

    # ---------------- DRAM parameters ----------------
    inp = nc.declare_dram_parameter("input", [T * B, D], FP32, isOutput=False)
    whh0t_d = nc.declare_dram_parameter("whh0t", [256, 1024], F32R, isOutput=False)
    wih0t_d = nc.declare_dram_parameter("wih0t", [256, 1024], F32R, isOutput=False)
    whh1t_d = nc.declare_dram_parameter("whh1t", [256, 1024], F32R, isOutput=False)
    wih1t_d = nc.declare_dram_parameter("wih1t", [256, 1024], F32R, isOutput=False)
    winitt_d = nc.declare_dram_parameter("winitt", [256, 256], F32R, isOutput=False)
    bias0_d = nc.declare_dram_parameter("bias0", [1, 1024], F32R, isOutput=False)
    bias1_d = nc.declare_dram_parameter("bias1", [1, 1024], F32R, isOutput=False)
    binit_d = nc.declare_dram_parameter("binit", [1, 256], F32R, isOutput=False)
    ones_d = nc.declare_dram_parameter("ones", [1, 512], F32R, isOutput=False)
    zeros_d = nc.declare_dram_parameter("zeros128", [128, 128], F32R, isOutput=False)
    id128_d = nc.declare_dram_parameter("id128", [128, 128], FP32, isOutput=False)
    h0t_init_d = nc.declare_dram_parameter("h0t_init", [256, 32], F32R, isOutput=False)
    h1t_init_d = nc.declare_dram_parameter("h1t_init", [256, 32], F32R, isOutput=False)
    c0_init_d = nc.declare_dram_parameter("c0_init", [32, 256], FP32, isOutput=False)
    c1_init_d = nc.declare_dram_parameter("c1_init", [32, 256], FP32, isOutput=False)
    fwd = nc.declare_dram_parameter("fwd", [T * B, H], FP32, isOutput=True)

    ctxs = []

    def sb(shape, dtype=None):
        cm = nc.sbuf_tensor(shape, dtype or FP32)
        t = cm.__enter__()
        ctxs.append(cm)
        return t

    def ps(shape, dtype=None):
        cm = nc.psum_tensor(shape, dtype or FP32)
        t = cm.__enter__()
        ctxs.append(cm)
        return t

    # ---------------- SBUF constants ----------------
    whh0t = [sb([128, 1024], F32R) for _ in range(2)]
    wih0t = [sb([128, 1024], F32R) for _ in range(2)]
    whh1t = [sb([128, 1024], F32R) for _ in range(2)]
    wih1t = [sb([128, 1024], F32R) for _ in range(2)]
    winitt = [sb([128, 256], F32R) for _ in range(2)]
    bias0 = sb([1, 1024], F32R)
    bias1 = sb([1, 1024], F32R)
    binit = sb([1, 256], F32R)
    ones = sb([1, 512], F32R)
    id128 = sb([128, 128])

    # ---------------- SBUF working buffers ----------------
    # input block (untransposed), per half-block ping-pong: 8 squares of
    # [128 rows, 256 dims] side by side
    inblk = [sb([128, 8 * 256]) for _ in range(2)]
    # input.T block: [256 dims -> 2 tiles][hb] of [128, 1024 (t,b) cols]
    inT = [[sb([128, 1024], F32R) for _ in range(2)] for _ in range(2)]  # [k][hb]
    # x.T block: same geometry
    xT = [[sb([128, 1024], F32R) for _ in range(2)] for _ in range(2)]  # [k][hb]
    # h.T group buffers [gb][k]: [128, 128] = 4 steps x 32 batch cols
    # (layer-0 only; feeds the bulk G1in matmul)
    h0t = [[sb([128, 128], F32R) for _ in range(2)] for _ in range(2)]  # [gb][k]
    # zero-padded stationary tiles for the step matmuls: Z[layer][pos][k] is
    # all-zero except column block `pos`, which holds h.T of step t with
    # (t+1) % 4 == pos.  The step matmul then targets all 128 PSUM
    # partitions at offset 0 (fp32r + tile_position column offsets is
    # rejected by the ISA), adding zero to the other steps' rows.
    Z = [[[sb([128, 128], F32R) for _ in range(2)] for _ in range(G4)]
         for _ in range(2)]
    # cell state ping-pong
    c0_ = [sb([32, 256]) for _ in range(2)]
    c1_ = [sb([32, 256]) for _ in range(2)]
    # elementwise scratch, per layer x parity
    sif = [[sb([32, 1024]) for _ in range(2)] for _ in range(2)]  # [l][p]
    fc = [[sb([32, 256]) for _ in range(2)] for _ in range(2)]
    ig = [[sb([32, 256]) for _ in range(2)] for _ in range(2)]
    tcc = [[sb([32, 256]) for _ in range(2)] for _ in range(2)]
    h0v = [sb([32, 256]) for _ in range(2)]  # layer0 hidden (untransposed)
    # output block per half-block: [32 batch parts, 32 steps * 256]
    outblk = [sb([32, HALF * 256]) for _ in range(2)]

    # ---------------- PSUM ----------------
    g0p = [ps([128, 1024]) for _ in range(2)]   # 4 banks
    g1p = ps([128, 1024])                        # 2 banks
    scrA = ps([128, 512])                        # 1 bank: h-transposes + input-T
    scrB = ps([128, 512])                        # 1 bank: x-MM chunks

    import concourse.tile as tile_mod

    with tile_mod.TileContext(nc) as tc:
        dma = nc.sync

        # ------------ constant + init loads ------------
        for k in range(2):
            dma.dma_start(whh0t[k][:, :], whh0t_d[128 * k:128 * (k + 1), :])
            dma.dma_start(wih0t[k][:, :], wih0t_d[128 * k:128 * (k + 1), :])
            dma.dma_start(whh1t[k][:, :], whh1t_d[128 * k:128 * (k + 1), :])
            dma.dma_start(wih1t[k][:, :], wih1t_d[128 * k:128 * (k + 1), :])
            dma.dma_start(winitt[k][:, :], winitt_d[128 * k:128 * (k + 1), :])
        dma.dma_start(bias0[:, :], bias0_d[:, :])
        dma.dma_start(bias1[:, :], bias1_d[:, :])
        dma.dma_start(binit[:, :], binit_d[:, :])
        dma.dma_start(ones[:, :], ones_d[:, :])
        dma.dma_start(id128[:, :], id128_d[:, :])
        # zero the padded stationary tiles once; non-pos blocks stay zero
        # (DMA from a DRAM zeros constant: DVE memset cannot emit f32r)
        for l in range(2):
            for pos in range(G4):
                for k in range(2):
                    dma.dma_start(Z[l][pos][k][:, :], zeros_d[:, :])
        # initial h.T: consumed by step t=0 from Z[l][0], column block 0
        for k in range(2):
            dma.dma_start(Z[0][0][k][:, 0:32], h0t_init_d[128 * k:128 * (k + 1), :])
            dma.dma_start(Z[1][0][k][:, 0:32], h1t_init_d[128 * k:128 * (k + 1), :])
        dma.dma_start(c0_[0][:, :], c0_init_d[:, :])
        dma.dma_start(c1_[0][:, :], c1_init_d[:, :])

        def emit_bulk_x(i, hb):
            """input DMA, transpose to input.T, x.T = W_init@input.T + b_init."""
            # one 3D DMA for the whole [1024 rows, 256] block
            src = inp[bass.ds(i + hb * (HALF * B), 1024), :].rearrange(
                "(r p) d -> p r d", p=128)
            dma.dma_start(inblk[hb][:, :].rearrange("p (r d) -> p r d", r=8), src)
            # 16 square transposes [128,128] -> inT
            for rr in range(8):
                for cdim in range(2):
                    sl = scrA[:, 128 + 128 * ((rr * 2 + cdim) % 2):
                              256 + 128 * ((rr * 2 + cdim) % 2)]
                    nc.tensor.transpose(
                        sl,
                        inblk[hb][:, 256 * rr + 128 * cdim:256 * rr + 128 * (cdim + 1)],
                        id128[:, :],
                    )
                    # evac: alternate ACT / DVE
                    dst = inT[cdim][hb][:, 128 * rr:128 * (rr + 1)]
                    if (rr + cdim) % 2 == 0:
                        nc.scalar.copy(dst, sl)
                    else:
                        nc.vector.tensor_copy(dst, sl)
            # x.T = W_init @ input.T + b_init, in [128,256] chunks
            for m in range(2):
                for cc in range(4):
                    out = scrB[:, 256 * (cc % 2):256 * (cc % 2 + 1)]
                    # bias ride: out = b_init[m-slice].T outer ones
                    nc.tensor.matmul(
                        out, binit[:, 128 * m:128 * (m + 1)],
                        ones[:, 0:256], start=True, stop=False,
                    )
                    for k in range(2):
                        nc.tensor.matmul(
                            out,
                            winitt[k][:, 128 * m:128 * (m + 1)],
                            inT[k][hb][:, 256 * cc:256 * (cc + 1)],
                            start=False, stop=(k == 1),
                        )
                    nc.scalar.copy(xT[m][hb][:, 256 * cc:256 * (cc + 1)], out)

        def emit_g0in(hb, g_loc, pp):
            """Bulk G0in for group: bias0 + x @ Wih0.T into g0p[pp]."""
            for c in range(2):
                out = g0p[pp][:, 512 * c:512 * (c + 1)]
                nc.tensor.matmul(out, ones[:, 0:128],
                                 bias0[:, 512 * c:512 * (c + 1)],
                                 start=True, stop=False)
                for k in range(2):
                    nc.tensor.matmul(
                        out,
                        xT[k][hb][:, 128 * g_loc:128 * (g_loc + 1)],
                        wih0t[k][:, 512 * c:512 * (c + 1)],
                        start=False, stop=(k == 1),
                    )

        def emit_g1in(pp):
            """Bulk G1in for group: bias1 + h0(group) @ Wih1.T into g1p."""
            for c in range(2):
                out = g1p[:, 512 * c:512 * (c + 1)]
                nc.tensor.matmul(out, ones[:, 0:128],
                                 bias1[:, 512 * c:512 * (c + 1)],
                                 start=True, stop=False)
                for k in range(2):
                    nc.tensor.matmul(
                        out,
                        h0t[pp][k][:, 0:128],
                        wih1t[k][:, 512 * c:512 * (c + 1)],
                        start=False, stop=(k == 1),
                    )

        def base_off(layer):
            return 0 if layer == 0 else 64

        def emit_step(layer, t_loc, hb, g_loc, j, pp):
            """One recurrent step for one layer."""
            p = t_loc % 2
            whht = whh0t if layer == 0 else whh1t
            gp = g0p[pp] if layer == 0 else g1p
            cc_ = c0_ if layer == 0 else c1_
            rows = slice(32 * j, 32 * (j + 1))

            # step matmul: h_{t-1}.T sits in column block j of the
            # zero-padded stationary Z[layer][j]; all other columns are 0,
            # so accumulating over all 128 partitions only updates rows j.
            for c in range(2):
                for k in range(2):
                    nc.tensor.matmul(
                        gp[:, 512 * c:512 * (c + 1)],
                        Z[layer][j][k][:, :],
                        whht[k][:, 512 * c:512 * (c + 1)],
                        start=False, stop=(k == 1), skip_group_check=True,
                    )
            # one sigmoid over all four gates; the g-gate's weights/bias
            # are pre-scaled by 2 on host so tanh(g) = 2*sig(2g) - 1 folds
            # into the DVE ops below.  Gate order [i f g o].
            AFt = AF
            s_ = sif[layer][p]
            nc.scalar.activation(s_[:, :], gp[rows, :], AFt.Sigmoid)
            # cell update: c = f*c + i*(2*sig(2g) - 1)
            c_prev = cc_[t_loc % 2]
            c_new = cc_[(t_loc + 1) % 2]
            nc.vector.tensor_mul(fc[layer][p][:, :], s_[:, 256:512], c_prev[:, :])
            nc.vector.scalar_tensor_tensor(
                ig[layer][p][:, :], s_[:, 512:768], 0.5, s_[:, 0:256],
                mybir.AluOpType.subtract, mybir.AluOpType.mult)
            nc.vector.scalar_tensor_tensor(
                c_new[:, :], ig[layer][p][:, :], 2.0, fc[layer][p][:, :],
                mybir.AluOpType.mult, mybir.AluOpType.add)
            nc.scalar.activation(tcc[layer][p][:, :], c_new[:, :], AFt.Tanh)
            # hidden
            if layer == 0:
                hv = h0v[p]
            else:
                hv = outblk[hb][:, 256 * (g_loc * G4 + j):256 * (g_loc * G4 + j + 1)]
            nc.vector.tensor_mul(hv[:, :], s_[:, 768:1024], tcc[layer][p][:, :])
            # transpose h -> h.T slices (2 halves of 128); the consumer of
            # h.T(t) is step t+1, which reads Z[layer][(t+1) % 4] block
            # (t+1) % 4.  Layer-0 h.T additionally feeds the bulk G1in
            # matmul via the contiguous group buffer h0t.
            nxt = (j + 1) % G4
            for k in range(2):
                sl = scrA[:, base_off(layer) + 32 * k:base_off(layer) + 32 * (k + 1)]
                nc.tensor.transpose(sl, hv[:, 128 * k:128 * (k + 1)], id128[0:32, 0:32])
                nc.vector.tensor_copy(Z[layer][nxt][k][:, 32 * nxt:32 * (nxt + 1)], sl)
                if layer == 0:
                    # group buffer for the bulk G1in matmul; gpsimd reads the
                    # SBUF Z block (gpsimd cannot read PSUM), keeping the
                    # copy off the ACT/DVE chains
                    nc.gpsimd.tensor_copy(
                        h0t[pp][k][:, 32 * j:32 * (j + 1)],
                        Z[0][nxt][k][:, 32 * nxt:32 * (nxt + 1)])

        def emit_out_dma(i, hb):
            src = outblk[hb][:, :].rearrange("b (t d) -> b t d", t=HALF)
            dst = fwd[bass.ds(i + hb * (HALF * B), HALF * B), :].rearrange(
                "(t b) d -> b t d", b=32)
            dma.dma_start(dst, src)

        def emit_body(i):
            # software pipeline: layer 1 lags layer 0 by one 4-step group so
            # the two dependence chains interleave on the engines.
            n_groups = BODY // G4  # 16
            for g in range(n_groups + 1):
                gl = g - 1          # lagged group for layer 1
                if g < n_groups:
                    hb = g // 8
                    g_loc = g % 8
                    if g_loc == 0:
                        emit_bulk_x(i, hb)
                    emit_g0in(hb, g_loc, g % 2)
                if gl >= 0:
                    # G1in(gl): h0t(gl) is complete; the WAR on g1p against
                    # L1(gl-1)'s reads resolved a full group ago
                    emit_g1in(gl % 2)
                for j in range(G4):
                    if g < n_groups:
                        emit_step(0, g * G4 + j, g // 8, g % 8, j, g % 2)
                    if gl >= 0:
                        emit_step(1, gl * G4 + j, gl // 8, gl % 8, j, gl % 2)
                if gl >= 0 and gl % 8 == 7:
                    emit_out_dma(i, gl // 8)

        if n_body == 1:
            emit_body(0)
        else:
            with tc.For_i(0, T * B, BODY * B) as i:
                emit_body(i)

    for cm in reversed(ctxs):
        cm.__exit__(None, None, None)

    nc.compile()
    return nc


def rne11(x):
    """Round fp32 to f32r: round-to-nearest-even keeping 11 mantissa bits."""
    xi = np.ascontiguousarray(x, np.float32).view(np.uint32).astype(np.uint64)
    shift = 12
    half = np.uint64(1 << (shift - 1))
    lsb = (xi >> np.uint64(shift)) & np.uint64(1)
    r = ((xi + half - np.uint64(1) + lsb) >> np.uint64(shift)) << np.uint64(shift)
    return (r & np.uint64(0xFFFFFFFF)).astype(np.uint32).view(np.float32).reshape(np.shape(x))


def prep_inputs(inputs, T):
    """Host-side input re-layout (cheap: weights only; input passed as-is)."""
    inp = np.ascontiguousarray(inputs["input"], dtype=np.float32)
    Wih = inputs["Wih"].astype(np.float32)
    Whh = inputs["Whh"].astype(np.float32)
    bih = inputs["bih"].astype(np.float32)
    bhh = inputs["bhh"].astype(np.float32)
    W_init = inputs["W_init"].astype(np.float32)
    b_init = inputs["b_init"].astype(np.float32)
    h0 = inputs["h0"].astype(np.float32)
    c0 = inputs["c0"].astype(np.float32)

    def g2(wt):
        w = np.ascontiguousarray(wt, np.float32).copy()
        w[:, 512:768] *= 2.0
        return w

    im = {
        "input": inp.reshape(T * B, D),
        "whh0t": rne11(g2(Whh[0].T)),
        "wih0t": rne11(g2(Wih[0].T)),
        "whh1t": rne11(g2(Whh[1].T)),
        "wih1t": rne11(g2(Wih[1].T)),
        "winitt": rne11(W_init.T),
        "bias0": rne11(g2((bih[0] + bhh[0]).reshape(1, 1024))),
        "bias1": rne11(g2((bih[1] + bhh[1]).reshape(1, 1024))),
        "binit": rne11(b_init.reshape(1, 256)),
        "ones": np.ones((1, 512), np.float32),
        "zeros128": np.zeros((128, 128), np.float32),
        "id128": np.eye(128, dtype=np.float32),
        "h0t_init": rne11(np.ascontiguousarray(h0[0].T)),
        "h1t_init": rne11(np.ascontiguousarray(h0[1].T)),
        "c0_init": np.ascontiguousarray(c0[0]),
        "c1_init": np.ascontiguousarray(c0[1]),
    }
    return im


def run_device(inputs, T, trace=False, repeats=0):
    """Run on hardware. trace/repeats: rerun the compiled NEFF to get a
    warm-execution wall time (NTFF profiling is unavailable under axon)."""
    import time
    from concourse import bass_utils

    nc = build(T)
    im = prep_inputs(inputs, T)
    res = bass_utils.run_bass_kernel_spmd(nc, [im], [0])
    times = []
    if trace or repeats:
        for _ in range(max(repeats, 3)):
            t0 = time.time()
            res = bass_utils.run_bass_kernel_spmd(nc, [im], [0])
            times.append(time.time() - t0)
        res.exec_time_ns = int(min(times) * 1e9)
    fwd = res.results[0]["fwd"].reshape(T, B, H)
    return fwd, res


def kernel(**inputs):
    T = inputs["input"].shape[0]
    fwd, _ = run_device(inputs, T)
    out = np.empty((T, B, 2 * H), dtype=np.float32)
    out[:, :, :H] = fwd
    out[:, :, H:] = fwd[-1][None]
    return out


if __name__ == "__main__":
    # quick CoreSim smoke test with small T
    import concourse.bass as bass  # noqa
    from concourse.bass_interp import CoreSim

    T = int(os.environ.get("SIM_T", "64"))
    rng = np.random.default_rng(0)
    k = 1.0 / np.sqrt(H)
    inputs = {
        "input": rng.standard_normal((T, B, D), dtype=np.float32),
        "W_init": rng.uniform(-k, k, (H, D)).astype(np.float32),
        "b_init": rng.uniform(-k, k, (H,)).astype(np.float32),
        "Wih": rng.uniform(-k, k, (2, 4 * H, H)).astype(np.float32),
        "Whh": rng.uniform(-k, k, (2, 4 * H, H)).astype(np.float32),
        "bih": rng.uniform(-k, k, (2, 4 * H)).astype(np.float32),
        "bhh": rng.uniform(-k, k, (2, 4 * H)).astype(np.float32),
        "h0": rng.uniform(-k, k, (2, B, H)).astype(np.float32),
        "c0": rng.uniform(-k, k, (2, B, H)).astype(np.float32),
    }

    # numpy reference
    def np_ref(inp):
        x_all = inp["input"]
        h = inp["h0"].copy()
        c = inp["c0"].copy()
        outs = []
        for t in range(T):
            x = x_all[t] @ inp["W_init"].T + inp["b_init"]
            for l in range(2):
                gates = x @ inp["Wih"][l].T + inp["bih"][l] + h[l] @ inp["Whh"][l].T + inp["bhh"][l]
                i_, f_, g_, o_ = np.split(gates, 4, axis=-1)
                i_ = 1 / (1 + np.exp(-i_)); f_ = 1 / (1 + np.exp(-f_))
                o_ = 1 / (1 + np.exp(-o_)); g_ = np.tanh(g_)
                c[l] = f_ * c[l] + i_ * g_
                h[l] = o_ * np.tanh(c[l])
                x = h[l]
            outs.append(h[1].copy())
        return np.stack(outs)

    expected = np_ref(inputs)

    nc = build(T)

    sim = CoreSim(nc)
    im = prep_inputs(inputs, T)
    for name, arr in im.items():
        sim.tensor(name)[:] = arr
    sim.simulate()
    got = sim.tensor("fwd").reshape(T, B, H)
    err = np.abs(got - expected).max() / (np.abs(expected).max() + 1e-9)
    print("SIM max-rel err:", err)
    print("sample got", got[0, 0, :4], "exp", expected[0, 0, :4])

